# revision 45
# baseline (speedup 1.0000x reference)
"""CrossAttention Trainium2 kernel (v4).

Full inputs -> shard over 8 cores (batch x head-group) -> Bass kernel ->
host gather (sum head-group partials per batch + bias).

Per-core layout (B=2 batches x 4 head-groups of 4 heads):
  xT    [1024, 2048]  x[b].T
  ctxT  [1024, 2048]  context[b].T
  wqT   [1024, 256]   Wq[rows(g)].T      (rows(g) = g*256 : (g+1)*256)
  wkT   [1024, 256]
  wvT   [1024, 256]
  woT   [256, 1024]   Wo[:, rows(g)].T
  out y [2048, 1024]  bf16 partial (sum over g gives batch output; bias on host)

v4 changes over v3 (246998ns baseline):
  - Score PSUM tiles are per-key-chunk [128, 1024] = [h0 512 | h1 512].
    The two 64-contraction score matmuls (tile_position (0,0)/(64,0))
    write different PSUM banks of the SAME tile, become slot-ready
    together, and execute concurrently on the PE's row groups (~2x the
    score throughput vs the v3 layout where the pair was staggered by
    one ACT period through the pool rotation).
  - m-loop runs per chunk (16 slots/iter), extras spread finer.
  - v-proj split per head-pair; loop-0 extras rebalanced so the ACT
    stream doesn't starve during the prologue.
  - reciprocal_approx_fast for the softmax denominators (DVE, ~5x).
  - y output in bf16 (halves output DMA); host accumulates in fp32.
"""
import numpy as np
import ml_dtypes

HEADS = 16
DIM_HEAD = 64
D_MODEL = 1024
N_CORES = 8


def build_nc(n_q=2048, n_kv=2048, d_model=1024, n_heads=4, d_head=64, nt=512):
    """Build the per-core Bass module."""
    import concourse.mybir as mybir
    import concourse.tile as tile
    from concourse import bacc

    FP32 = mybir.dt.float32
    BF16 = mybir.dt.bfloat16
    EXP = mybir.ActivationFunctionType.Exp
    P = 128

    inner = n_heads * d_head          # 256
    ND = d_model // P                 # 8 contraction chunks
    NI = inner // P                   # 2 head-pairs
    NNT = n_q // nt                   # 4 query tiles
    NMC = n_kv // P                   # 16 key chunks
    NJ = d_model // nt                # 2 output col tiles

    nc = bacc.Bacc(None, target_bir_lowering=False, debug=False)

    # Inputs are pre-arranged on the host into the exact SBUF layout
    # ([partition, chunk, col]) so every input DMA is a contiguous
    # per-partition copy (128 descriptors) instead of a ~1k-segment
    # scatter -- cuts both descriptor-issue time and transfer time.
    xT = nc.dram_tensor("xT", [P, ND * n_q], BF16, kind="ExternalInput")
    ctxT = nc.dram_tensor("ctxT", [P, ND * n_kv], BF16, kind="ExternalInput")
    wqT = nc.dram_tensor("wqT", [P, ND * inner], BF16, kind="ExternalInput")
    wkT = nc.dram_tensor("wkT", [P, ND * inner], BF16, kind="ExternalInput")
    wvT = nc.dram_tensor("wvT", [P, ND * inner], BF16, kind="ExternalInput")
    woT = nc.dram_tensor("woT", [P, NI * d_model], BF16, kind="ExternalInput")
    y = nc.dram_tensor("y", [n_q, d_model], BF16, kind="ExternalOutput")

    xT_r = xT.ap().rearrange("p (c n) -> p c n", c=ND)     # [128, ND, n_q]
    ctxT_r = ctxT.ap().rearrange("p (c m) -> p c m", c=ND)
    wqT_r = wqT.ap().rearrange("p (c i) -> p c i", c=ND)
    wkT_r = wkT.ap().rearrange("p (c i) -> p c i", c=ND)
    wvT_r = wvT.ap().rearrange("p (c i) -> p c i", c=ND)
    woT_r = woT.ap().rearrange("p (c j) -> p c j", c=NI)   # [128, NI, d_model]

    scale = float(d_head) ** -0.5

    with tile.TileContext(nc) as tc:
        with (
            tc.tile_pool(name="persist", bufs=1) as persist,
            tc.tile_pool(name="vpool", bufs=NMC) as vpool,
        ):
            # ---------------- persistent tiles ----------------
            qT_sb = [persist.tile([P, n_q], BF16, tag=f"qT{i}", name=f"qT{i}")
                     for i in range(NI)]
            kT_sb = [persist.tile([P, n_kv], BF16, tag=f"kT{i}", name=f"kT{i}")
                     for i in range(NI)]
            woT_sb = persist.tile([P, NI, d_model], BF16, tag="woT")
            wq_sb = persist.tile([P, ND, inner], BF16, tag="wq")
            wk_sb = persist.tile([P, ND, inner], BF16, tag="wk")
            wv_sb = persist.tile([P, ND, inner], BF16, tag="wv")
            xq_sb = persist.tile([P, ND, n_q], BF16, tag="xq")
            ck_sb = persist.tile([P, ND, n_kv], BF16, tag="ck")

            # warm the ACT exp table before any real work
            warm = persist.tile([P, 8], FP32, tag="warm")
            nc.vector.memset(warm[:], 0.0)
            nc.scalar.activation(warm[0:1, 0:8], warm[0:1, 0:8], EXP)

            # E: bf16 broadcast matrix (row0 -> out rows 0..63, row64 ->
            # 64..127); bf16 keeps the E-matmul single-pass (fp32 matmuls
            # run as two half-speed LOW/HIGH passes).
            scratch = persist.tile([P, nt], BF16, tag="scratch")
            nc.vector.memset(scratch[:], 0.0)
            E_sb = persist.tile([P, P], BF16, tag="E")
            nc.vector.tensor_copy(E_sb[:], scratch[:, 0:P])
            ones_sc = persist.tile([P, 64], BF16, tag="ones_sc")
            nc.vector.memset(ones_sc[:], 1.0)
            nc.vector.tensor_copy(E_sb[0:1, 0:64], ones_sc[0:1, :])
            nc.vector.tensor_copy(E_sb[64:65, 64:128], ones_sc[64:65, :])
            # ssb rows 1..63 stay 1.0 so the [0:65] reciprocal is finite
            ssb_p = persist.tile([P, nt], FP32, tag="ssb_p")
            nc.vector.memset(ssb_p[:], 1.0)
            # rs32: fp32 recip result; rs16: bf16 cast fed to the E-matmul
            # (rows 65..127 stay 0 -- the E-matmul reads all partitions)
            rs32 = persist.tile([P, nt], FP32, tag="rs32")
            nc.vector.memset(rs32[:], 0.0)
            rs16 = persist.tile([P, nt], BF16, tag="rs16")
            nc.vector.memset(rs16[:], 0.0)
            # v tiles: per m-chunk [128, heads, 65]; col 64 is the ones
            # column (softmax denominator trick)
            v_sb = [vpool.tile([P, n_heads, 65], BF16, tag="vsb", name=f"vsb{m}")
                    for m in range(NMC)]
            for m in range(NMC):
                nc.vector.memset(v_sb[m][:, :, 64:65], 1.0)

            # ------------- input DMAs (k-proj critical path first) ------
            # Ordered so the first-exp chain (wk, ck[0:256] -> kproj;
            # xq0+wq -> qproj; scores) is gated by as little DMA transfer
            # as possible.
            nc.sync.dma_start(wk_sb[:, :, 0:P], wkT_r[:, :, 0:P])
            nc.sync.dma_start(ck_sb[:, :, 0:2 * P], ctxT_r[:, :, 0:2 * P])
            nc.sync.dma_start(wq_sb[:, :, 0:P], wqT_r[:, :, 0:P])
            nc.sync.dma_start(xq_sb[:, :, 0:nt], xT_r[:, :, 0:nt])
            nc.sync.dma_start(wk_sb[:, :, P:2 * P], wkT_r[:, :, P:2 * P])
            nc.sync.dma_start(wv_sb[:], wvT_r[:, :, :])
            nc.sync.dma_start(ck_sb[:, :, 2 * P:nt], ctxT_r[:, :, 2 * P:nt])
            nc.sync.dma_start(wq_sb[:, :, P:2 * P], wqT_r[:, :, P:2 * P])
            nc.sync.dma_start(ck_sb[:, :, nt:4 * nt], ctxT_r[:, :, nt:4 * nt])
            nc.sync.dma_start(xq_sb[:, :, nt:4 * nt], xT_r[:, :, nt:4 * nt])
            nc.sync.dma_start(woT_sb[:], woT_r[:, :, :])

            # ---------------- stages 2-4 ----------------
            with (
                tc.tile_pool(name="psq", bufs=2, space="PSUM") as psq,
                tc.tile_pool(name="upool", bufs=2, space="PSUM") as upool,
                tc.tile_pool(name="mpool", bufs=2, space="PSUM") as mpool,
                tc.tile_pool(name="expp", bufs=6) as expp,
                tc.tile_pool(name="s2sb", bufs=2) as s2sb,
                tc.tile_pool(name="apool", bufs=4) as apool,
                tc.tile_pool(name="ypool", bufs=6) as ypool,
            ):
                A_tiles = {}
                U_tiles = {}
                qp_tiles = {}

                # -------- PE warm-up + stage-1 priming (in mpool) --------
                # Dummy matmuls on already-memset tiles keep the PE busy
                # while the first input DMAs land, ramping it to the full
                # 2.4GHz p-state (cold PE runs matmuls ~1.7x slower).
                wps = mpool.tile([P, nt], FP32, tag="m", name="warm_ps")
                for r in range(10):
                    nc.tensor.matmul(wps[:], E_sb[:], scratch[:],
                                     start=(r == 0), stop=(r == 9))
                # kT[0] chunks 0-3 (critical path for the first scores)
                ps = mpool.tile([P, nt], FP32, tag="m", name="s1_k01")
                for d in range(ND):
                    nc.tensor.matmul(
                        ps[:, 0:2 * P], wk_sb[:, d, 0:P], ck_sb[:, d, 0:2 * P],
                        start=(d == 0), stop=(d == ND - 1))
                nc.vector.tensor_copy(kT_sb[0][:, 0:2 * P], ps[:, 0:2 * P])
                # second warm burst bridges the gap until the xq DMA lands
                # so the q-proj runs at full clock
                wps2 = mpool.tile([P, nt], FP32, tag="m", name="warm_ps2")
                for r in range(8):
                    nc.tensor.matmul(wps2[:], E_sb[:], scratch[:],
                                     start=(r == 0), stop=(r == 7))
                # qT[0] first n-tile
                ps = mpool.tile([P, nt], FP32, tag="m", name="s1_q0")
                for d in range(ND):
                    nc.tensor.matmul(
                        ps[:], wq_sb[:, d, 0:P], xq_sb[:, d, 0:nt],
                        start=(d == 0), stop=(d == ND - 1))
                nc.vector.tensor_copy(qT_sb[0][:, 0:nt], ps[:])
                # qT[1] first n-tile (needed by loop(0,hp1))
                ps = mpool.tile([P, nt], FP32, tag="m", name="s1_q1")
                for d in range(ND):
                    nc.tensor.matmul(
                        ps[:], wq_sb[:, d, P:2 * P], xq_sb[:, d, 0:nt],
                        start=(d == 0), stop=(d == ND - 1))
                nc.vector.tensor_copy(qT_sb[1][:, 0:nt], ps[:])
                # v chunks 0-1, all heads (first AVs of loop(0,hp0))
                for m in range(2):
                    psv = mpool.tile([P, n_heads, d_head], FP32, tag="m",
                                     name=f"s1_v{m}")
                    for d in range(ND):
                        nc.tensor.matmul(
                            psv[:], ck_sb[:, d, m * P:(m + 1) * P],
                            wv_sb[:, d, :],
                            start=(d == 0), stop=(d == ND - 1))
                    nc.vector.tensor_copy(v_sb[m][:, :, 0:64], psv[:])
                # kT[0] chunks 2-3 (ck[256:512] lands after the q inputs;
                # needed by the third score chunk, well after first-exp)
                ps = mpool.tile([P, nt], FP32, tag="m", name="s1_k23")
                for d in range(ND):
                    nc.tensor.matmul(
                        ps[:, 0:2 * P], wk_sb[:, d, 0:P],
                        ck_sb[:, d, 2 * P:4 * P],
                        start=(d == 0), stop=(d == ND - 1))
                nc.vector.tensor_copy(kT_sb[0][:, 2 * P:4 * P], ps[:, 0:2 * P])

                def tail_dve(n, hp, use_act=False):
                    """Evacuate U, reciprocal of denominators (DVE).

                    The reciprocal chain (E-matmul input) runs first so the
                    PE-side tail consumers unblock as early as possible;
                    the U casts (only needed by the A-mul) follow.
                    use_act routes the two big U casts through the scalar
                    engine -- only worthwhile in the final tail where ACT
                    is otherwise idle.
                    """
                    U0, U1 = U_tiles[(n, hp)]
                    nc.vector.tensor_copy(ssb_p[0:1, :], U0[64:65, :])
                    nc.vector.tensor_copy(ssb_p[64:65, :], U1[64:65, :])
                    with nc.allow_low_precision(reason="softmax recip"):
                        nc.vector.reciprocal_approx_fast(
                            rs32[0:65, :], ssb_p[0:65, :])
                        nc.vector.tensor_copy(rs16[0:65, :], rs32[0:65, :])
                    usb = s2sb.tile([P, nt], BF16, tag="usb",
                                    name=f"usb{n}_{hp}")
                    if use_act:
                        nc.scalar.copy(usb[0:64, :], U0[0:64, :])
                        nc.scalar.copy(usb[64:128, :], U1[0:64, :])
                    else:
                        nc.vector.tensor_copy(usb[0:64, :], U0[0:64, :])
                        nc.vector.tensor_copy(usb[64:128, :], U1[0:64, :])
                    U_tiles[(n, hp)] = (usb,)

                def tail_emm(n, hp):
                    bps = mpool.tile([P, nt], FP32, tag="m",
                                     name=f"bps{n}_{hp}")
                    nc.tensor.matmul(bps[:], E_sb[:], rs16[:],
                                     start=True, stop=True)
                    U_tiles[(n, hp)] = U_tiles[(n, hp)] + (bps,)

                def tail_amul(n, hp):
                    usb, bps = U_tiles.pop((n, hp))
                    A = apool.tile([P, nt], BF16, tag="A", name=f"A{n}_{hp}")
                    A_tiles[(n, hp)] = A
                    nc.vector.tensor_mul(A[:], usb[:], bps[:])

                def qproj_piece(n, i, ph, per=2):
                    """`per` accumulating MMs of q-proj chain i; cast at
                    the final phase."""
                    nsl = slice(n * nt, (n + 1) * nt)
                    if ph == 0:
                        qp_tiles[(n, i)] = mpool.tile(
                            [P, nt], FP32, tag="m", name=f"qp{n}_{i}")
                    ps = qp_tiles[(n, i)]
                    for d in range(per * ph, per * ph + per):
                        nc.tensor.matmul(
                            ps[:],
                            wq_sb[:, d, i * P:(i + 1) * P],
                            xq_sb[:, d, nsl],
                            start=(d == 0), stop=(d == ND - 1))
                    if per * ph + per == ND:
                        nc.vector.tensor_copy(qT_sb[i][:, nsl], ps[:])
                        del qp_tiles[(n, i)]

                def outproj_mm(n, g):
                    """First half of an out-proj group (c=0 matmul)."""
                    q, j = g // NJ, g % NJ
                    qsl = slice(q * P, (q + 1) * P)
                    jsl = slice(j * nt, (j + 1) * nt)
                    yps = mpool.tile([P, nt], FP32, tag="m",
                                     name=f"yp{n}_{q}_{j}")
                    nc.tensor.matmul(
                        yps[:], A_tiles[(n, 0)][:, qsl], woT_sb[:, 0, jsl],
                        start=True, stop=False)
                    qp_tiles[("y", n, g)] = yps

                def outproj_fin(n, g, use_act=False):
                    """Second half: c=1 matmul + cast + DMA."""
                    q, j = g // NJ, g % NJ
                    qsl = slice(q * P, (q + 1) * P)
                    jsl = slice(j * nt, (j + 1) * nt)
                    yps = qp_tiles.pop(("y", n, g))
                    nc.tensor.matmul(
                        yps[:], A_tiles[(n, 1)][:, qsl], woT_sb[:, 1, jsl],
                        start=False, stop=True)
                    ysb = ypool.tile([P, nt], BF16, tag="ysb")
                    if use_act:
                        nc.scalar.copy(ysb[:], yps[:])
                    else:
                        nc.vector.tensor_copy(ysb[:], yps[:])
                    nc.sync.dma_start(
                        y.ap()[n * nt + q * P:n * nt + (q + 1) * P, jsl],
                        ysb[:])

                def vproj_chunk(m):
                    """v-proj for key chunk m, all heads."""
                    psv = mpool.tile([P, n_heads, d_head], FP32, tag="m",
                                     name=f"vp{m}")
                    for d in range(ND):
                        nc.tensor.matmul(
                            psv[:], ck_sb[:, d, m * P:(m + 1) * P],
                            wv_sb[:, d, :],
                            start=(d == 0), stop=(d == ND - 1))
                    nc.vector.tensor_copy(v_sb[m][:, :, 0:64], psv[:])

                def kproj_tile(i, mt):
                    msl = slice(mt * nt, (mt + 1) * nt)
                    ps = mpool.tile([P, nt], FP32, tag="m", name=f"kp{i}_{mt}")
                    for d in range(ND):
                        nc.tensor.matmul(
                            ps[:],
                            wk_sb[:, d, i * P:(i + 1) * P],
                            ck_sb[:, d, msl],
                            start=(d == 0), stop=(d == ND - 1))
                    nc.vector.tensor_copy(kT_sb[i][:, msl], ps[:])

                def outproj_group(n, g, use_act=False):
                    q, j = g // NJ, g % NJ
                    qsl = slice(q * P, (q + 1) * P)
                    jsl = slice(j * nt, (j + 1) * nt)
                    yps = mpool.tile([P, nt], FP32, tag="m",
                                     name=f"yp{n}_{q}_{j}")
                    for c in range(NI):
                        nc.tensor.matmul(
                            yps[:], A_tiles[(n, c)][:, qsl],
                            woT_sb[:, c, jsl],
                            start=(c == 0), stop=(c == NI - 1))
                    ysb = ypool.tile([P, nt], BF16, tag="ysb")
                    if use_act:
                        nc.scalar.copy(ysb[:], yps[:])
                    else:
                        nc.vector.tensor_copy(ysb[:], yps[:])
                    nc.sync.dma_start(
                        y.ap()[n * nt + q * P:n * nt + (q + 1) * P, jsl],
                        ysb[:])

                prefetched = {}

                def emit_scores(n, hp, m):
                    """One key chunk, both heads, one [128,1024] psum tile.

                    h0 -> cols 0:512 (psum bank a), h1 -> cols 512:1024
                    (bank b): the row-tiled pair runs concurrently.
                    """
                    nsl = slice(n * nt, (n + 1) * nt)
                    msl = slice(m * P, (m + 1) * P)
                    pq = psq.tile([P, 2 * nt], FP32, tag="psq")
                    nc.tensor.matmul(
                        pq[:, 0:nt],
                        kT_sb[hp][0:64, msl],
                        qT_sb[hp][0:64, nsl],
                        start=True, stop=True, tile_position=(0, 0))
                    nc.tensor.matmul(
                        pq[:, nt:2 * nt],
                        kT_sb[hp][64:128, msl],
                        qT_sb[hp][64:128, nsl],
                        start=True, stop=True, tile_position=(64, 0))
                    ex = expp.tile([P, 2 * nt], BF16, tag="ex")
                    nc.scalar.activation(ex[:], pq[:], EXP, scale=scale)
                    return ex

                def mloop(n, hp, extras, nxt=None):
                    h0, h1 = 2 * hp, 2 * hp + 1
                    U0 = upool.tile([P, nt], FP32, tag="U", name=f"U0_{n}_{hp}")
                    U1 = upool.tile([P, nt], FP32, tag="U", name=f"U1_{n}_{hp}")
                    U_tiles[(n, hp)] = (U0, U1)
                    # scores are emitted one chunk ahead of their AV: the
                    # AV(m) matmuls sit in the PE queue waiting on ACT(m),
                    # and the in-order queue would otherwise head-of-line
                    # block scores(m+1) behind them, adding ~0.1us to every
                    # ACT period.
                    if (n, hp) in prefetched:
                        ex_cur = prefetched.pop((n, hp))
                    else:
                        ex_cur = emit_scores(n, hp, 0)
                    for m in range(NMC):
                        if m + 1 < NMC:
                            ex_next = emit_scores(n, hp, m + 1)
                        else:
                            ex_next = None
                        if m == NMC - 1 and nxt is not None:
                            # prefetch the next loop's first chunk so ACT
                            # rolls straight across the boundary
                            prefetched[nxt] = emit_scores(nxt[0], nxt[1], 0)
                        # extras run on the tensor queue while the AV
                        # matmuls below would be waiting on the activations
                        for fn in extras.get(m, []):
                            fn()
                        first = (m == 0)
                        last = (m == NMC - 1)
                        nc.tensor.matmul(
                            U0[0:65, :], v_sb[m][:, h0, :], ex_cur[:, 0:nt],
                            start=first, stop=last)
                        nc.tensor.matmul(
                            U1[0:65, :], v_sb[m][:, h1, :],
                            ex_cur[:, nt:2 * nt],
                            start=first, stop=last)
                        ex_cur = ex_next

                for n in range(NNT):
                    # ---- hp0 m-loop ----
                    ex0_sched = {}
                    if n == 0:
                        # prime: v chunks 2-15 (hp0) staggered ~2 ahead of
                        # their AV; kT[0] tiles 1-3 before chunks 4/8/12;
                        # kT[1] tile 0 before loop(0,hp1); v chunks 0-3
                        # (hp1) at the end for loop(0,hp1)'s first AVs.
                        for m in range(2, NMC):
                            ex0_sched.setdefault(m - 2, []).append(
                                lambda m=m: vproj_chunk(m))
                        ex0_sched.setdefault(1, []).append(
                            lambda: kproj_tile(0, 1))
                        ex0_sched.setdefault(3, []).append(
                            lambda: kproj_tile(1, 0))
                        ex0_sched.setdefault(5, []).append(
                            lambda: kproj_tile(0, 2))
                        ex0_sched.setdefault(9, []).append(
                            lambda: kproj_tile(0, 3))
                    else:
                        # tail(n-1,hp1) + out-proj(n-1) + qproj(n, i=1);
                        # keep slots 12-15 light so the loop boundary
                        # (prefetch + next loop's first chunks) doesn't
                        # starve ACT.
                        ex0_sched = {
                            0: [lambda: tail_dve(n - 1, 1)],
                            2: [lambda: tail_emm(n - 1, 1)],
                            4: [lambda: tail_amul(n - 1, 1),
                                lambda: outproj_group(n - 1, 0)],
                            5: [lambda: outproj_group(n - 1, 1)],
                            6: [lambda: outproj_group(n - 1, 2)],
                            7: [lambda: outproj_group(n - 1, 3)],
                            8: [lambda: outproj_group(n - 1, 4)],
                            9: [lambda: outproj_group(n - 1, 5)],
                            10: [lambda: outproj_group(n - 1, 6)],
                            11: [lambda: outproj_group(n - 1, 7)],
                        }
                        for ph in range(4):
                            ex0_sched.setdefault(10 + ph, []).append(
                                lambda ph=ph: qproj_piece(n, 1, ph))
                    mloop(n, 0, ex0_sched, nxt=(n, 1))

                    # ---- hp1 m-loop ----
                    ex1_sched = {
                        0: [lambda: tail_dve(n, 0)],
                        2: [lambda: tail_emm(n, 0)],
                        4: [lambda: tail_amul(n, 0)],
                    }

                    if n == 0:
                        # remaining prologue work
                        ex1_sched.setdefault(1, []).append(
                            lambda: kproj_tile(1, 1))
                        ex1_sched.setdefault(5, []).append(
                            lambda: kproj_tile(1, 2))
                        ex1_sched.setdefault(9, []).append(
                            lambda: kproj_tile(1, 3))
                    if n + 1 < NNT:
                        for ph in range(8):
                            ex1_sched.setdefault(6 + ph, []).append(
                                lambda ph=ph: qproj_piece(n + 1, 0, ph,
                                                          per=1))
                    mloop(n, 1, ex1_sched,
                          nxt=(n + 1, 0) if n + 1 < NNT else None)

                # final tail + out-proj, serial (ACT is idle here, so the
                # U casts go through the scalar engine)
                tail_dve(NNT - 1, 1, use_act=True)
                tail_emm(NNT - 1, 1)
                tail_amul(NNT - 1, 1)
                for g in range(8):
                    outproj_group(NNT - 1, g, use_act=(g % 2 == 1))

    nc.compile()
    return nc


def _chunked(arrT):
    """[C*128, X] -> [128, C*X]: SBUF-layout pre-arrangement (partition,
    contraction-chunk, col) so device DMAs are contiguous per partition."""
    d, X = arrT.shape
    C = d // 128
    return np.ascontiguousarray(
        arrT.reshape(C, 128, X).transpose(1, 0, 2).reshape(128, C * X))


def shard_inputs(x, context, Wq, Wk, Wv, Wo):
    """Per-core input dicts: core c -> (batch c//4, head-group c%4)."""
    in_maps = []
    for c in range(N_CORES):
        b, g = c // 4, c % 4
        rows = slice(g * 256, (g + 1) * 256)
        bf = ml_dtypes.bfloat16
        in_maps.append({
            "xT": _chunked(x[b].T).astype(bf),
            "ctxT": _chunked(context[b].T).astype(bf),
            "wqT": _chunked(Wq[rows].T).astype(bf),
            "wkT": _chunked(Wk[rows].T).astype(bf),
            "wvT": _chunked(Wv[rows].T).astype(bf),
            "woT": _chunked(Wo[:, rows].T).astype(bf),
        })
    return in_maps


_CACHE = {}


def _get_nc():
    if "nc" not in _CACHE:
        _CACHE["nc"] = build_nc()
    return _CACHE["nc"]


def kernel(x, context, Wq, Wk, Wv, Wo, bo, _trace=False):
    from concourse.bass_utils import run_bass_kernel_spmd

    x = np.asarray(x, dtype=np.float32)
    context = np.asarray(context, dtype=np.float32)
    in_maps = shard_inputs(x, context,
                           np.asarray(Wq, np.float32), np.asarray(Wk, np.float32),
                           np.asarray(Wv, np.float32), np.asarray(Wo, np.float32))
    nc = _get_nc()
    res = run_bass_kernel_spmd(nc, in_maps, core_ids=list(range(N_CORES)),
                               trace=_trace)
    B, N, _ = x.shape
    out = np.zeros((B, N, D_MODEL), dtype=np.float32)
    for c in range(N_CORES):
        out[c // 4] += np.asarray(res.results[c]["y"], dtype=np.float32)
    out += np.asarray(bo, np.float32)[None, None, :]
    if _trace:
        _CACHE["last_results"] = res
    return out
